# revision 13
# baseline (speedup 1.0000x reference)
"""Causal self-attention (B=2, T=2048, C=1024, H=16) on 8 trn2 NeuronCores.

Sharding: core i handles batch b = i // 4 and head-group hg = i % 4
(4 heads each). Data-parallel over B, tensor-parallel over heads:
each core computes q/k/v for its 4 heads, full causal attention locally,
and a partial projection out = y_heads @ W_proj[rows]; the host sums the
4 partials per batch. No collectives.

All compute in bf16 (inputs converted host-side; PSUM accumulates fp32).

Layout: transposed space, no on-chip transposes:
  - host passes xT = x[b].T  [C, T] bf16
  - qT/kT [d, T] straight out of the qkv matmul; per m, qt[m]/kt[m]
    [128, T] stack head 2m in partitions 0:64 and head 2m+1 in 64:128
  - scores: per k-tile, TWO concurrent K=64 matmuls via PE row tiling
    (head A in array rows 0-63, head B in rows 64-127; auto
    tile_position from base partitions) into one [128, 1024] PSUM tile
    [A | B] -> ONE exp per k-tile on ACT (bf16 out)
  - mask: multiply diagonal 128x128 bands by 0/1 mask post-exp (DVE)
  - y/denom: psy[65, 512] accumulates va_aug.T @ pt over k-tiles
    (va col 64 = ones)
  - divide: one [65,512] copy out of PSUM, reciprocal, gpsimd
    partition_broadcast, multiply into yt (bf16)
  - proj: yt as stationary bf16 (FWL), out partial [T, C] fp32,
    PSUM->SBUF copy on the Pool engine, DMA out
Schedule: fine-grained interleave of qkv/proj units into attention
k-tile yield points so PE and ACT stay busy together.
"""

import sys

import numpy as np

sys.path.insert(0, "/opt/trn_rl_repo")

B, T, C = 2, 2048, 1024
N_HEAD = 16
D = C // N_HEAD          # 64
HPC = N_HEAD // 4        # 4 heads per core
CS = HPC * D             # 256 = per-core slice width of q/k/v
NCHUNK = C // 128        # 8 contraction chunks over C
NT = T // 128            # 16 row tiles
NQ = T // 512            # 4 query tiles of 512
SCALE = 1.0 / np.sqrt(D)

_CACHE = {}


def _build():
    from collections import deque

    import concourse.bacc as bacc
    import concourse.mybir as mybir
    import concourse.tile as tile

    F32 = mybir.dt.float32
    BF16 = mybir.dt.bfloat16

    nc = bacc.Bacc("TRN2", target_bir_lowering=False, debug=False, num_devices=8)

    xT = nc.dram_tensor("xT", [C, T], BF16, kind="ExternalInput").ap()
    wq = nc.dram_tensor("wq", [128, NCHUNK * CS], BF16, kind="ExternalInput").ap()
    wk = nc.dram_tensor("wk", [128, NCHUNK * CS], BF16, kind="ExternalInput").ap()
    wv = nc.dram_tensor("wv", [128, NCHUNK * CS], BF16, kind="ExternalInput").ap()
    wp = nc.dram_tensor("wp", [128, 2 * C], BF16, kind="ExternalInput").ap()
    mask = nc.dram_tensor("mask", [128, 128], BF16, kind="ExternalInput").ap()
    out = nc.dram_tensor("out", [T, C], BF16, kind="ExternalOutput").ap()

    with tile.TileContext(nc) as tc:
        with (
            tc.tile_pool(name="persist", bufs=1) as pp,
            tc.tile_pool(name="consts", bufs=1) as cp,
            tc.tile_pool(name="xw", bufs=1) as xw,
            tc.tile_pool(name="xs", bufs=2) as xsp,
            tc.tile_pool(name="pt", bufs=6) as ptp,
            tc.tile_pool(name="sm", bufs=2) as smp,
            tc.tile_pool(name="po", bufs=4) as pop,
            tc.tile_pool(name="psm", bufs=2, space="PSUM") as psm_p,
            tc.tile_pool(name="psb", bufs=2, space="PSUM") as psb_p,
            tc.tile_pool(name="psy", bufs=2, space="PSUM") as psy_p,
        ):
            # ---------------- persistent SBUF ----------------
            # qt/kt[m]: head 2m in partitions 0:64, head 2m+1 in 64:128
            qt = [pp.tile([128, T], BF16, name=f"qt{m}", tag=f"qt{m}")
                  for m in range(2)]
            kt = [pp.tile([128, T], BF16, name=f"kt{m}", tag=f"kt{m}")
                  for m in range(2)]
            yt = [pp.tile([128, T], BF16, name=f"yt{m}", tag=f"yt{m}")
                  for m in range(2)]
            # v_aug per row-tile: [128, 4 heads, 65] (col 64 = ones)
            va = [pp.tile([128, HPC, D + 1], BF16, name=f"va{t}", tag=f"va{t}")
                  for t in range(NT)]
            mk = cp.tile([128, 128], BF16, tag="mask")
            wpt = cp.tile([128, 2, C], BF16, tag="wp")
            ones_bf = cp.tile([128, HPC], BF16, tag="ones")
            warm_src = cp.tile([128, 1], F32, tag="warmsrc")
            warm = cp.tile([128, 1], F32, tag="warm")

            # ---------------- DMAs, critical-path first ----------------
            wqt = xw.tile([128, NCHUNK, CS], BF16, tag="wq")
            wkt = xw.tile([128, NCHUNK, CS], BF16, tag="wk")
            wvt = xw.tile([128, NCHUNK, CS], BF16, tag="wv")
            # 4 slices share 2 slots: slice ns+2's DMA waits until slice ns
            # is consumed (automatic WAR dep via the shared tag)
            xts = [xsp.tile([128, NCHUNK, 512], BF16, name=f"xt{ns}",
                            tag="xt") for ns in range(NQ)]

            def dma_x(ns):
                for c in range(NCHUNK):
                    nc.sync.dma_start(
                        xts[ns][:, c],
                        xT[c * 128:(c + 1) * 128, ns * 512:(ns + 1) * 512],
                    )

            nc.sync.dma_start(wqt[:].rearrange("p c n -> p (c n)"), wq[:])
            dma_x(0)
            nc.sync.dma_start(wkt[:].rearrange("p c n -> p (c n)"), wk[:])
            nc.sync.dma_start(wvt[:].rearrange("p c n -> p (c n)"), wv[:])
            dma_x(1)
            nc.sync.dma_start(mk[:], mask[:])
            dma_x(2)
            nc.sync.dma_start(wpt[:].rearrange("p c n -> p (c n)"), wp[:])
            dma_x(3)

            nc.gpsimd.memset(ones_bf[:], 1.0)
            nc.gpsimd.memset(warm_src[:], 1.0)
            # warm the ACT exp table early (off the critical path)
            nc.scalar.activation(warm[:], warm_src[:],
                                 mybir.ActivationFunctionType.Exp, scale=1.0)

            # ---------------- qkv units ----------------
            def qk_unit(ns, m, which):
                sl = slice(ns * 512, (ns + 1) * 512)
                w_all = wqt if which == "q" else wkt
                ps = psb_p.tile([128, 512], F32, tag="psb", name="psqk")
                for c in range(NCHUNK):
                    nc.tensor.matmul(
                        ps[:],
                        w_all[:, c, m * 128:(m + 1) * 128],
                        xts[ns][:, c, :],
                        start=(c == 0),
                        stop=(c == NCHUNK - 1),
                    )
                if which == "q":
                    # ACT is near-idle in qkv-heavy stretches
                    nc.scalar.copy(qt[m][:, sl], ps[:])
                else:
                    nc.vector.tensor_copy(kt[m][:, sl], ps[:])

            def v_unit(ns, t):
                ps = psb_p.tile([128, CS], F32, tag="psb", name="psv")
                for c in range(NCHUNK):
                    nc.tensor.matmul(
                        ps[:],
                        xts[ns][:, c, (t % 4) * 128:(t % 4 + 1) * 128],
                        wvt[:, c, :],
                        start=(c == 0),
                        stop=(c == NCHUNK - 1),
                    )
                nc.vector.tensor_copy(
                    va[t][:, :, 0:D],
                    ps[:].rearrange("p (h d) -> p h d", h=HPC),
                )
                nc.vector.tensor_copy(va[t][:, :, D], ones_bf[:])

            # ---------------- attention ----------------
            def divide(h, j, psy):
                # reciprocal_approx_fast requires a partition-0 input on HW
                hq, ho = h // 2, (h % 2) * 64
                den = smp.tile([1, 512], F32, tag="den", name="den")
                nc.vector.tensor_copy(den[:], psy[D:D + 1, :])
                rec = smp.tile([1, 512], F32, tag="rec", name="rec")
                nc.vector.reciprocal_approx_fast(rec[:], den[:])
                bc = smp.tile([D, 512], F32, tag="bc", name="bc")
                nc.gpsimd.partition_broadcast(bc[:], rec[:])
                nc.vector.tensor_mul(
                    yt[hq][ho:ho + 64, j * 512:(j + 1) * 512],
                    psy[0:D, :],
                    bc[:],
                )

            def attention(j, hp):
                nkb = 4 * (j + 1)
                psyA = psy_p.tile([D + 1, 512], F32, tag="psy", name="psyA")
                psyB = psy_p.tile([D + 1, 512], F32, tag="psy", name="psyB")
                q0_sl = j * 512
                pending = [None]

                def y_acc(kb):
                    pkb, pq0, ppt = pending[0]
                    assert pkb == kb
                    nc.tensor.matmul(
                        psyA[:, pq0:512], va[kb][:, 2 * hp, :],
                        ppt[:, 0, pq0:512],
                        start=(kb == 0), stop=(kb == nkb - 1),
                    )
                    nc.tensor.matmul(
                        psyB[:, pq0:512], va[kb][:, 2 * hp + 1, :],
                        ppt[:, 1, pq0:512],
                        start=(kb == 0), stop=(kb == nkb - 1),
                    )

                for kb in range(nkb):
                    di = kb - 4 * j
                    q0 = 128 * di if di > 0 else 0
                    ksl = slice(kb * 128, (kb + 1) * 128)
                    qsl = slice(q0_sl + q0, q0_sl + 512)
                    pss = psm_p.tile([128, 2, 512], F32, tag="psm", name="pss")
                    # two K=64 matmuls, concurrent via PE row tiling
                    nc.tensor.matmul(
                        pss[:, 0, q0:512], kt[hp][0:64, ksl], qt[hp][0:64, qsl],
                        start=True, stop=True,
                    )
                    nc.tensor.matmul(
                        pss[:, 1, q0:512], kt[hp][64:128, ksl],
                        qt[hp][64:128, qsl],
                        start=True, stop=True,
                    )
                    pt = ptp.tile([128, 2, 512], BF16, tag="pt", name="pt")
                    # one exp for both heads: strided AP over the written spans
                    nc.scalar.activation(
                        pt[:, :, q0:512], pss[:, :, q0:512],
                        mybir.ActivationFunctionType.Exp, scale=float(SCALE),
                    )
                    if di >= 0:
                        # zero the upper triangle of the diagonal band (Pool)
                        nc.gpsimd.tensor_mul(
                            pt[:, 0, q0:q0 + 128], pt[:, 0, q0:q0 + 128], mk[:]
                        )
                        nc.gpsimd.tensor_mul(
                            pt[:, 1, q0:q0 + 128], pt[:, 1, q0:q0 + 128], mk[:]
                        )
                    yield
                    # y lags one k-tile so the PE queue never head-blocks on
                    # the exp of the k-tile it just issued
                    if kb > 0:
                        y_acc(kb - 1)
                    pending[0] = (kb, q0, pt)
                    yield
                y_acc(nkb - 1)
                divide(2 * hp, j, psyA)
                divide(2 * hp + 1, j, psyB)

            def proj_unit(j, t):
                for nb in range(2):
                    ps = psb_p.tile([128, 512], F32, tag="psb", name="pso")
                    for cc in range(2):
                        nc.tensor.matmul(
                            ps[:],
                            yt[cc][:, t * 128:(t + 1) * 128],
                            wpt[:, cc, nb * 512:(nb + 1) * 512],
                            start=(cc == 0),
                            stop=(cc == 1),
                        )
                    ot = pop.tile([128, 512], BF16, tag="po", name="po")
                    nc.vector.tensor_copy(ot[:], ps[:])
                    nc.sync.dma_start(
                        out[t * 128:(t + 1) * 128,
                            nb * 512:(nb + 1) * 512],
                        ot[:],
                    )

            # ---------------- interleaved schedule ----------------
            def b_units(ns):
                units = []
                for m in range(2):
                    units.append(lambda ns=ns, m=m: qk_unit(ns, m, "q"))
                    units.append(lambda ns=ns, m=m: qk_unit(ns, m, "k"))
                for t in range(4 * ns, 4 * ns + 4):
                    units.append(lambda ns=ns, t=t: v_unit(ns, t))
                return units

            for u in b_units(0):
                u()
            bq = deque()            # qkv units for segments 1..3
            for ns in range(1, NQ):
                for u in b_units(ns):
                    bq.append((ns, u))
            pq = deque()            # proj units, unlocked per j-block
            b_emitted = 1           # segments fully emitted

            def emit_filler():
                nonlocal b_emitted
                if bq:
                    ns, u = bq.popleft()
                    u()
                    if not bq or bq[0][0] != ns:
                        b_emitted = ns + 1
                elif pq:
                    pq.popleft()()

            tasks = [(j, hp) for j in range(NQ) for hp in range(2)]
            for j, hp in tasks:
                while b_emitted <= j:
                    emit_filler()
                for _ in attention(j, hp):
                    emit_filler()
                if hp == 1:
                    for t in range(4 * j, 4 * j + 4):
                        pq.append(lambda j=j, t=t: proj_unit(j, t))
            while bq or pq:
                emit_filler()

    nc.compile()
    return nc


def _causal_mask():
    kk = np.arange(128)[:, None]
    cc = np.arange(128)[None, :]
    return (cc >= kk).astype(np.float32)


def _get_nc():
    if "nc" not in _CACHE:
        _CACHE["nc"] = _build()
    return _CACHE["nc"]


def _run(x, W_qkv, W_proj, trace=False, trace_cores=None):
    import ml_dtypes
    from concourse.bass_utils import run_bass_kernel_spmd

    BF = ml_dtypes.bfloat16
    x = np.asarray(x, dtype=np.float32)
    W_qkv = np.asarray(W_qkv, dtype=np.float32)
    W_proj = np.asarray(W_proj, dtype=np.float32)

    nc = _get_nc()
    mask = _causal_mask().astype(BF)
    in_maps = []
    for core in range(8):
        b, hg = core // 4, core % 4
        sl = slice(hg * CS, (hg + 1) * CS)

        def warr(w):  # [K, N] -> [128, (K//128)*N] chunk-major per partition
            return np.ascontiguousarray(
                w.reshape(w.shape[0] // 128, 128, -1)
                .transpose(1, 0, 2).reshape(128, -1).astype(BF)
            )

        in_maps.append({
            "xT": np.ascontiguousarray(x[b].T.astype(BF)),
            "wq": warr(W_qkv[:, sl]),
            "wk": warr(W_qkv[:, C + hg * CS:C + (hg + 1) * CS]),
            "wv": warr(W_qkv[:, 2 * C + hg * CS:2 * C + (hg + 1) * CS]),
            "wp": warr(W_proj[sl, :]),
            "mask": mask,
        })

    res = run_bass_kernel_spmd(
        nc, in_maps, list(range(8)), trace=trace, trace_cores=trace_cores
    )
    outp = np.zeros((B, T, C), dtype=np.float32)
    for core in range(8):
        outp[core // 4] += res.results[core]["out"].astype(np.float32)
    return outp, res


def kernel(x, W_qkv, W_proj):
    outp, _ = _run(x, W_qkv, W_proj)
    return outp


# revision 15
# speedup vs baseline: 1.5391x; 1.5391x over previous
"""Causal self-attention (B=2, T=2048, C=1024, H=16) on 8 trn2 NeuronCores.

Sharding: core i handles batch b = i // 4 and head-group hg = i % 4
(4 heads each). Data-parallel over B, tensor-parallel over heads:
each core computes q/k/v for its 4 heads, full causal attention locally,
and a partial projection out = y_heads @ W_proj[rows]; the host sums the
4 partials per batch. No collectives.

All compute in bf16 (inputs converted host-side; PSUM accumulates fp32).

Layout: transposed space, no on-chip transposes:
  - host passes xT = x[b].T  [C, T] bf16
  - qT/kT [d, T] straight out of the qkv matmul; per m, qt[m]/kt[m]
    [128, T] stack head 2m in partitions 0:64 and head 2m+1 in 64:128
  - scores: per k-tile, TWO concurrent K=64 matmuls via PE row tiling
    (head A in array rows 0-63, head B in rows 64-127; auto
    tile_position from base partitions) into one [128, 1024] PSUM tile
    [A | B] -> ONE exp per k-tile on ACT (bf16 out)
  - mask: multiply diagonal 128x128 bands by 0/1 mask post-exp (DVE)
  - y/denom: psy[65, 512] accumulates va_aug.T @ pt over k-tiles
    (va col 64 = ones)
  - divide: one [65,512] copy out of PSUM, reciprocal, gpsimd
    partition_broadcast, multiply into yt (bf16)
  - proj: yt as stationary bf16 (FWL), out partial [T, C] fp32,
    PSUM->SBUF copy on the Pool engine, DMA out
Schedule: fine-grained interleave of qkv/proj units into attention
k-tile yield points so PE and ACT stay busy together.
"""

import sys

import numpy as np

sys.path.insert(0, "/opt/trn_rl_repo")

B, T, C = 2, 2048, 1024
N_HEAD = 16
D = C // N_HEAD          # 64
HPC = N_HEAD // 4        # 4 heads per core
CS = HPC * D             # 256 = per-core slice width of q/k/v
NCHUNK = C // 128        # 8 contraction chunks over C
NT = T // 128            # 16 row tiles
NQ = T // 512            # 4 query tiles of 512
SCALE = 1.0 / np.sqrt(D)

_CACHE = {}


def _build():
    from collections import deque

    import concourse.bacc as bacc
    import concourse.mybir as mybir
    import concourse.tile as tile

    F32 = mybir.dt.float32
    BF16 = mybir.dt.bfloat16

    nc = bacc.Bacc("TRN2", target_bir_lowering=False, debug=False, num_devices=8)

    xT = nc.dram_tensor("xT", [C, T], BF16, kind="ExternalInput").ap()
    wq = nc.dram_tensor("wq", [128, NCHUNK * CS], BF16, kind="ExternalInput").ap()
    wk = nc.dram_tensor("wk", [128, NCHUNK * CS], BF16, kind="ExternalInput").ap()
    wv = nc.dram_tensor("wv", [128, NCHUNK * CS], BF16, kind="ExternalInput").ap()
    wp = nc.dram_tensor("wp", [128, 2 * C], BF16, kind="ExternalInput").ap()
    mask = nc.dram_tensor("mask", [128, 128], BF16, kind="ExternalInput").ap()
    out = nc.dram_tensor("out", [T, C], BF16, kind="ExternalOutput").ap()

    with tile.TileContext(nc) as tc:
        with (
            tc.tile_pool(name="persist", bufs=1) as pp,
            tc.tile_pool(name="consts", bufs=1) as cp,
            tc.tile_pool(name="xw", bufs=1) as xw,
            tc.tile_pool(name="xs", bufs=2) as xsp,
            tc.tile_pool(name="pt", bufs=6) as ptp,
            tc.tile_pool(name="sm", bufs=2) as smp,
            tc.tile_pool(name="po", bufs=4) as pop,
            tc.tile_pool(name="psm", bufs=2, space="PSUM") as psm_p,
            tc.tile_pool(name="psb", bufs=2, space="PSUM") as psb_p,
            tc.tile_pool(name="psy", bufs=2, space="PSUM") as psy_p,
        ):
            # ---------------- persistent SBUF ----------------
            # qt/kt[m]: head 2m in partitions 0:64, head 2m+1 in 64:128
            qt = [pp.tile([128, T], BF16, name=f"qt{m}", tag=f"qt{m}")
                  for m in range(2)]
            kt = [pp.tile([128, T], BF16, name=f"kt{m}", tag=f"kt{m}")
                  for m in range(2)]
            yt = [pp.tile([128, T], BF16, name=f"yt{m}", tag=f"yt{m}")
                  for m in range(2)]
            # v_aug per row-tile: [128, 4 heads, 65] (col 64 = ones)
            va = [pp.tile([128, HPC, D + 1], BF16, name=f"va{t}", tag=f"va{t}")
                  for t in range(NT)]
            mk = cp.tile([128, 128], BF16, tag="mask")
            wpt = cp.tile([128, 2, C], BF16, tag="wp")
            ones_bf = cp.tile([128, HPC], BF16, tag="ones")
            warm_src = cp.tile([128, 1], F32, tag="warmsrc")
            warm = cp.tile([128, 1], F32, tag="warm")

            # ---------------- DMAs, critical-path first ----------------
            wqt = xw.tile([128, NCHUNK, CS], BF16, tag="wq")
            wkt = xw.tile([128, NCHUNK, CS], BF16, tag="wk")
            wvt = xw.tile([128, NCHUNK, CS], BF16, tag="wv")
            # 4 slices share 2 slots: slice ns+2's DMA waits until slice ns
            # is consumed (automatic WAR dep via the shared tag)
            xts = [xsp.tile([128, NCHUNK, 512], BF16, name=f"xt{ns}",
                            tag="xt") for ns in range(NQ)]

            def dma_x(ns):
                for c in range(NCHUNK):
                    nc.sync.dma_start(
                        xts[ns][:, c],
                        xT[c * 128:(c + 1) * 128, ns * 512:(ns + 1) * 512],
                    )

            nc.sync.dma_start(wqt[:].rearrange("p c n -> p (c n)"), wq[:])
            dma_x(0)
            nc.sync.dma_start(wkt[:].rearrange("p c n -> p (c n)"), wk[:])
            nc.sync.dma_start(wvt[:].rearrange("p c n -> p (c n)"), wv[:])
            dma_x(1)
            nc.sync.dma_start(mk[:], mask[:])
            dma_x(2)
            nc.sync.dma_start(wpt[:].rearrange("p c n -> p (c n)"), wp[:])
            dma_x(3)

            nc.gpsimd.memset(ones_bf[:], 1.0)
            nc.gpsimd.memset(warm_src[:], 1.0)
            # warm the ACT exp table early (off the critical path)
            nc.scalar.activation(warm[:], warm_src[:],
                                 mybir.ActivationFunctionType.Exp, scale=1.0)

            # ---------------- qkv units ----------------
            def qk_unit(ns, m, which):
                sl = slice(ns * 512, (ns + 1) * 512)
                w_all = wqt if which == "q" else wkt
                ps = psb_p.tile([128, 512], F32, tag="psb", name="psqk")
                for c in range(NCHUNK):
                    nc.tensor.matmul(
                        ps[:],
                        w_all[:, c, m * 128:(m + 1) * 128],
                        xts[ns][:, c, :],
                        start=(c == 0),
                        stop=(c == NCHUNK - 1),
                    )
                if which == "q":
                    # ACT is near-idle in qkv-heavy stretches
                    nc.scalar.copy(qt[m][:, sl], ps[:])
                else:
                    nc.vector.tensor_copy(kt[m][:, sl], ps[:])

            def v_unit(ns, t):
                ps = psb_p.tile([128, CS], F32, tag="psb", name="psv")
                for c in range(NCHUNK):
                    nc.tensor.matmul(
                        ps[:],
                        xts[ns][:, c, (t % 4) * 128:(t % 4 + 1) * 128],
                        wvt[:, c, :],
                        start=(c == 0),
                        stop=(c == NCHUNK - 1),
                    )
                nc.vector.tensor_copy(
                    va[t][:, :, 0:D],
                    ps[:].rearrange("p (h d) -> p h d", h=HPC),
                )
                nc.vector.tensor_copy(va[t][:, :, D], ones_bf[:])

            # ---------------- attention ----------------
            def divide(h, j, psy):
                # copy y and den out fast (releases the psy bank); the slow
                # recip/broadcast chain then runs off the critical path.
                # reciprocal_approx_fast requires a partition-0 input on HW.
                hq, ho = h // 2, (h % 2) * 64
                yu = smp.tile([D, 512], F32, tag="yu", name="yu")
                nc.vector.tensor_copy(yu[:], psy[0:D, :])
                den = smp.tile([1, 512], F32, tag="den", name="den")
                nc.vector.tensor_copy(den[:], psy[D:D + 1, :])
                rec = smp.tile([1, 512], F32, tag="rec", name="rec")
                nc.vector.reciprocal_approx_fast(rec[:], den[:])
                bc = smp.tile([D, 512], F32, tag="bc", name="bc")
                nc.gpsimd.partition_broadcast(bc[:], rec[:])
                nc.vector.tensor_mul(
                    yt[hq][ho:ho + 64, j * 512:(j + 1) * 512],
                    yu[:],
                    bc[:],
                )

            def attention(j, hp):
                nkb = 4 * (j + 1)
                psyA = psy_p.tile([D + 1, 512], F32, tag="psy", name="psyA")
                psyB = psy_p.tile([D + 1, 512], F32, tag="psy", name="psyB")
                q0_sl = j * 512
                pending = [None]

                def y_acc(kb):
                    pkb, pq0, ppt = pending[0]
                    assert pkb == kb
                    nc.tensor.matmul(
                        psyA[:, pq0:512], va[kb][:, 2 * hp, :],
                        ppt[:, 0, pq0:512],
                        start=(kb == 0), stop=(kb == nkb - 1),
                    )
                    nc.tensor.matmul(
                        psyB[:, pq0:512], va[kb][:, 2 * hp + 1, :],
                        ppt[:, 1, pq0:512],
                        start=(kb == 0), stop=(kb == nkb - 1),
                    )

                for kb in range(nkb):
                    di = kb - 4 * j
                    q0 = 128 * di if di > 0 else 0
                    ksl = slice(kb * 128, (kb + 1) * 128)
                    qsl = slice(q0_sl + q0, q0_sl + 512)
                    pss = psm_p.tile([128, 2, 512], F32, tag="psm", name="pss")
                    # two K=64 matmuls, concurrent via PE row tiling
                    nc.tensor.matmul(
                        pss[:, 0, q0:512], kt[hp][0:64, ksl], qt[hp][0:64, qsl],
                        start=True, stop=True,
                    )
                    nc.tensor.matmul(
                        pss[:, 1, q0:512], kt[hp][64:128, ksl],
                        qt[hp][64:128, qsl],
                        start=True, stop=True,
                    )
                    pt = ptp.tile([128, 2, 512], BF16, tag="pt", name="pt")
                    # one exp for both heads: strided AP over the written spans
                    nc.scalar.activation(
                        pt[:, :, q0:512], pss[:, :, q0:512],
                        mybir.ActivationFunctionType.Exp, scale=float(SCALE),
                    )
                    if di >= 0:
                        # zero the upper triangle of the diagonal band.
                        # NB: must NOT run on gpsimd — mixing native tensor
                        # ops with partition_broadcast there forces a ~5us
                        # microcode library swap per alternation.
                        nc.vector.tensor_mul(
                            pt[:, 0, q0:q0 + 128], pt[:, 0, q0:q0 + 128], mk[:]
                        )
                        nc.vector.tensor_mul(
                            pt[:, 1, q0:q0 + 128], pt[:, 1, q0:q0 + 128], mk[:]
                        )
                    yield
                    # y lags one k-tile so the PE queue never head-blocks on
                    # the exp of the k-tile it just issued
                    if kb > 0:
                        y_acc(kb - 1)
                    pending[0] = (kb, q0, pt)
                    yield
                y_acc(nkb - 1)
                divide(2 * hp, j, psyA)
                divide(2 * hp + 1, j, psyB)

            def proj_unit(j, t):
                for nb in range(2):
                    ps = psb_p.tile([128, 512], F32, tag="psb", name="pso")
                    for cc in range(2):
                        nc.tensor.matmul(
                            ps[:],
                            yt[cc][:, t * 128:(t + 1) * 128],
                            wpt[:, cc, nb * 512:(nb + 1) * 512],
                            start=(cc == 0),
                            stop=(cc == 1),
                        )
                    ot = pop.tile([128, 512], BF16, tag="po", name="po")
                    nc.vector.tensor_copy(ot[:], ps[:])
                    nc.sync.dma_start(
                        out[t * 128:(t + 1) * 128,
                            nb * 512:(nb + 1) * 512],
                        ot[:],
                    )

            # ---------------- interleaved schedule ----------------
            def b_units(ns):
                units = []
                for m in range(2):
                    units.append(lambda ns=ns, m=m: qk_unit(ns, m, "q"))
                    units.append(lambda ns=ns, m=m: qk_unit(ns, m, "k"))
                for t in range(4 * ns, 4 * ns + 4):
                    units.append(lambda ns=ns, t=t: v_unit(ns, t))
                return units

            for u in b_units(0):
                u()
            bq = deque()            # qkv units for segments 1..3
            for ns in range(1, NQ):
                for u in b_units(ns):
                    bq.append((ns, u))
            pq = deque()            # proj units, unlocked per j-block
            b_emitted = 1           # segments fully emitted

            def emit_filler():
                nonlocal b_emitted
                if bq:
                    ns, u = bq.popleft()
                    u()
                    if not bq or bq[0][0] != ns:
                        b_emitted = ns + 1
                elif pq:
                    pq.popleft()()

            tasks = [(j, hp) for j in range(NQ) for hp in range(2)]
            for j, hp in tasks:
                while b_emitted <= j:
                    emit_filler()
                for _ in attention(j, hp):
                    emit_filler()
                if hp == 1:
                    for t in range(4 * j, 4 * j + 4):
                        pq.append(lambda j=j, t=t: proj_unit(j, t))
            while bq or pq:
                emit_filler()

    nc.compile()
    return nc


def _causal_mask():
    kk = np.arange(128)[:, None]
    cc = np.arange(128)[None, :]
    return (cc >= kk).astype(np.float32)


def _get_nc():
    if "nc" not in _CACHE:
        _CACHE["nc"] = _build()
    return _CACHE["nc"]


def _run(x, W_qkv, W_proj, trace=False, trace_cores=None):
    import ml_dtypes
    from concourse.bass_utils import run_bass_kernel_spmd

    BF = ml_dtypes.bfloat16
    x = np.asarray(x, dtype=np.float32)
    W_qkv = np.asarray(W_qkv, dtype=np.float32)
    W_proj = np.asarray(W_proj, dtype=np.float32)

    nc = _get_nc()
    mask = _causal_mask().astype(BF)
    in_maps = []
    for core in range(8):
        b, hg = core // 4, core % 4
        sl = slice(hg * CS, (hg + 1) * CS)

        def warr(w):  # [K, N] -> [128, (K//128)*N] chunk-major per partition
            return np.ascontiguousarray(
                w.reshape(w.shape[0] // 128, 128, -1)
                .transpose(1, 0, 2).reshape(128, -1).astype(BF)
            )

        in_maps.append({
            "xT": np.ascontiguousarray(x[b].T.astype(BF)),
            "wq": warr(W_qkv[:, sl]),
            "wk": warr(W_qkv[:, C + hg * CS:C + (hg + 1) * CS]),
            "wv": warr(W_qkv[:, 2 * C + hg * CS:2 * C + (hg + 1) * CS]),
            "wp": warr(W_proj[sl, :]),
            "mask": mask,
        })

    res = run_bass_kernel_spmd(
        nc, in_maps, list(range(8)), trace=trace, trace_cores=trace_cores
    )
    outp = np.zeros((B, T, C), dtype=np.float32)
    for core in range(8):
        outp[core // 4] += res.results[core]["out"].astype(np.float32)
    return outp, res


def kernel(x, W_qkv, W_proj):
    outp, _ = _run(x, W_qkv, W_proj)
    return outp
